# revision 3
# baseline (speedup 1.0000x reference)
"""Trainium2 Bass kernel for nn_Enhancement_11819749999257.

Computes: 3x (1x1-conv MLP w/ BN+relu) feature embeddings + soft scatter of
per-joint features onto a 7x7 grid ("bone projection"), concatenated.

Full output: (256, 4736, 7, 7) f32 = 237 MB  -> memory(write)-bound.

Strategy (pure data parallel over batch, 8 cores x 32 batch items):
  - n = b_local*74 + j  flattens (batch item, joint). The per-core output
    (32, 4736, 49) is contiguous as rows n: out[n, c*49+s]. Rows are
    processed in 19 chunks of 128 partitions; each chunk's store is a
    [128 part x 12544 B contiguous] DMA (~1.57 MB) -> near peak HBM BW.
  - MLP: w1/w2 are 64x64; BN (eval) folded into per-channel scale/bias on
    host. PE matmuls: y1 = relu(scale*(w1 @ x) + bias) computed once for
    all n; per chunk F = (y1_chunk.T @ w2.T) + b2 lands directly in
    [n-partition, c-free] layout (no transpose needed).
  - Grid weights: W[n, s] = relu(1 - sqrt((gy_s+eps-u_n)^2 + (gx_s+eps-v_n)^2))
    via ACT Square (per-partition bias = -uv), DVE add, ACT Sqrt, ACT Relu.
  - Scatter: OUT[n, c*49+s] = F[n, c] * W[n, s] -- one DVE tensor_tensor
    mult with stride-0 broadcast APs ([128,64,1] x [128,1,49]).
"""

import numpy as np

import concourse.bass as bass
import concourse.mybir as mybir
from concourse import bacc, bass_utils
from concourse.tile import TileContext

F32 = mybir.dt.float32
AF = mybir.ActivationFunctionType
ALU = mybir.AluOpType

N_CORES = 8
B = 256
B_LOC = B // N_CORES      # 32
J = 74                    # 21 + 21 + 32 joints, concat order r, l, o
C = 64
S = 7
S2 = S * S                # 49
NLOC = B_LOC * J          # 2368 rows per core
P = 128
NCHUNK = (NLOC + P - 1) // P   # 19 (last chunk has 64 valid rows)
NPAD = NCHUNK * P         # 2432
OUT_COLS = C * S2         # 3136
EPS = 1.0e-6


def _build_module():
    nc = bacc.Bacc(None)
    names = {}
    with TileContext(nc) as tc:
        with tc.tile_pool(name="dram", bufs=1, space="DRAM") as dram:
            xall = dram.tile((C, NPAD), F32, kind="ExternalInput", name="xall")
            uvr = dram.tile((P, 2 * NCHUNK), F32, kind="ExternalInput", name="uvr")
            gyc = dram.tile((P, S2), F32, kind="ExternalInput", name="gyc")
            gxc = dram.tile((P, S2), F32, kind="ExternalInput", name="gxc")
            w1t = dram.tile((C, C), F32, kind="ExternalInput", name="w1t")
            w2t = dram.tile((C, C), F32, kind="ExternalInput", name="w2t")
            sc1 = dram.tile((C, 1), F32, kind="ExternalInput", name="sc1")
            bi1 = dram.tile((C, 1), F32, kind="ExternalInput", name="bi1")
            b2r = dram.tile((P, C), F32, kind="ExternalInput", name="b2r")
            out = dram.tile((NLOC, OUT_COLS), F32, kind="ExternalOutput", name="out")
            for ap in (xall, uvr, gyc, gxc, w1t, w2t, sc1, bi1, b2r, out):
                names[ap.tensor.name.rsplit("_", 1)[0]] = ap.tensor.name

            with (
                tc.tile_pool(name="consts", bufs=1) as cpool,
                tc.tile_pool(name="ps_a", bufs=2, space="PSUM") as ps_a,
                tc.tile_pool(name="ps_b", bufs=3, space="PSUM") as ps_b,
                tc.tile_pool(name="work", bufs=3) as wpool,
                tc.tile_pool(name="outs", bufs=4) as opool,
            ):
                # ---- constants / activations staging ----
                w1t_sb = cpool.tile_from(w1t[:])
                w2t_sb = cpool.tile_from(w2t[:])
                gyc_sb = cpool.tile_from(gyc[:])
                gxc_sb = cpool.tile_from(gxc[:])
                sc1_sb = cpool.tile_from(sc1[:])
                bi1_sb = cpool.tile_from(bi1[:])
                b2r_sb = cpool.tile_from(b2r[:])
                uvr_sb = cpool.tile_from(uvr[:])
                x_sb = cpool.tile_from(xall[:])

                # nuv[p, 2k+i] = -(uv_raw+1)*3.5  (per-partition biases)
                nuv_sb = cpool.tile((P, 2 * NCHUNK), F32)
                nc.scalar.activation(nuv_sb[:], uvr_sb, AF.Copy, bias=-3.5, scale=-3.5)

                # ---- phase A: y1 = relu(scale*(w1 @ x) + bias) over all n ----
                y1_sb = cpool.tile((C, NPAD), F32)
                NA = 512
                for a0 in range(0, NPAD, NA):
                    aw = min(NA, NPAD - a0)
                    ps1 = ps_a.tile((C, NA), F32, tag="ps1")
                    nc.tensor.matmul(
                        ps1[:, :aw], lhsT=w1t_sb, rhs=x_sb[:, a0 : a0 + aw]
                    )
                    nc.scalar.activation(
                        y1_sb[:, a0 : a0 + aw], ps1[:, :aw], AF.Relu,
                        bias=bi1_sb, scale=sc1_sb,
                    )

                # ---- phase B: per 128-row chunk ----
                for k in range(NCHUNK):
                    rows = min(P, NLOC - k * P)
                    # F = y1_chunk.T @ w2t + b2  -> [128 rows(n), 64 (c)]
                    psf = ps_b.tile((P, C), F32, tag="psf")
                    nc.tensor.matmul(
                        psf[:], lhsT=y1_sb[:, k * P : (k + 1) * P], rhs=w2t_sb
                    )
                    f_sb = wpool.tile((P, C), F32, tag="f")
                    nc.vector.tensor_tensor(f_sb[:], psf[:], b2r_sb, ALU.add)

                    # W[n, s] = relu(1 - sqrt((gy-u)^2 + (gx-v)^2))
                    sq0 = wpool.tile((P, S2), F32, tag="sq0")
                    sq1 = wpool.tile((P, S2), F32, tag="sq1")
                    nc.scalar.activation(
                        sq0[:], gyc_sb, AF.Square, bias=nuv_sb[:, 2 * k : 2 * k + 1]
                    )
                    nc.scalar.activation(
                        sq1[:], gxc_sb, AF.Square, bias=nuv_sb[:, 2 * k + 1 : 2 * k + 2]
                    )
                    ssum = wpool.tile((P, S2), F32, tag="ssum")
                    nc.vector.tensor_tensor(ssum[:], sq0[:], sq1[:], ALU.add)
                    dist = wpool.tile((P, S2), F32, tag="dist")
                    nc.scalar.activation(dist[:], ssum[:], AF.Sqrt)
                    w_sb = wpool.tile((P, S2), F32, tag="w")
                    nc.scalar.activation(w_sb[:], dist[:], AF.Relu, bias=1.0, scale=-1.0)

                    # OUT[n, c*49+s] = F[n, c] * W[n, s]
                    o_sb = opool.tile((P, OUT_COLS), F32, tag="o")
                    f_bc, w_bc = bass.broadcast_tensor_aps(
                        f_sb[:, :, None], w_sb[:, None, :]
                    )
                    o_3d = o_sb.rearrange("p (c s) -> p c s", s=S2)
                    nc.vector.tensor_tensor(o_3d, f_bc, w_bc, ALU.mult)

                    nc.sync.dma_start(
                        out=out[k * P : k * P + rows, :], in_=o_sb[:rows, :]
                    )
    nc.compile()
    return nc, names


_CACHE = {}


def _get_module():
    if "nc" not in _CACHE:
        _CACHE["nc"], _CACHE["names"] = _build_module()
    return _CACHE["nc"], _CACHE["names"]


def _prep_inputs(j2d_r, j2d_l, kp2d_o, feat_r, feat_l, feat_o,
                 w1, b1, bn_gamma, bn_beta, bn_mean, bn_var, w2, b2):
    """Host-side marshaling: shard batch, pack layouts. Returns in_maps."""
    f32 = np.float32
    # grid: grid[s] = (x[s%7], x[s//7]) with x = arange(7)+0.5
    x = (np.arange(S, dtype=f32) + 0.5)
    gy = np.tile(x, S) + EPS            # gy[s] = x[s%7] + eps
    gx = np.repeat(x, S) + EPS          # gx[s] = x[s//7] + eps
    gyc = np.broadcast_to(gy, (P, S2)).copy()
    gxc = np.broadcast_to(gx, (P, S2)).copy()

    scale = (bn_gamma.astype(f32) / np.sqrt(bn_var.astype(f32) + np.float32(1e-5)))
    bias1 = (b1.astype(f32) - bn_mean.astype(f32)) * scale + bn_beta.astype(f32)
    w1t = np.ascontiguousarray(w1.astype(f32).T)
    w2t = np.ascontiguousarray(w2.astype(f32).T)
    b2r = np.broadcast_to(b2.astype(f32), (P, C)).copy()

    xcat = np.concatenate([feat_r, feat_l, feat_o], axis=2).astype(f32)  # (B,64,74)
    jcat = np.concatenate([j2d_r, j2d_l, kp2d_o], axis=1).astype(f32)   # (B,74,2)

    shared = dict(
        gyc=gyc, gxc=gxc, w1t=w1t, w2t=w2t,
        sc1=scale.reshape(C, 1).copy(), bi1=bias1.reshape(C, 1).copy(), b2r=b2r,
    )
    in_maps = []
    for c in range(N_CORES):
        sl = slice(c * B_LOC, (c + 1) * B_LOC)
        # xall[c_ch, n] = xcat[b', c_ch, j], n = b'*74+j ; pad n to 2432
        xc = np.transpose(xcat[sl], (1, 0, 2)).reshape(C, NLOC)
        xall = np.zeros((C, NPAD), f32)
        xall[:, :NLOC] = xc
        # uvr[p, 2k+i] = jcat[n=128k+p, i]
        jc = np.zeros((NPAD, 2), f32)
        jc[:NLOC] = jcat[sl].reshape(NLOC, 2)
        uvr = np.ascontiguousarray(
            jc.reshape(NCHUNK, P, 2).transpose(1, 0, 2).reshape(P, 2 * NCHUNK)
        )
        in_maps.append(dict(shared, xall=xall, uvr=uvr))
    return in_maps


def kernel_with_results(trace=False, **inputs):
    nc, names = _get_module()
    in_maps_l = _prep_inputs(**inputs)
    in_maps = [{names[k]: v for k, v in m.items()} for m in in_maps_l]
    res = bass_utils.run_bass_kernel_spmd(
        nc, in_maps, core_ids=list(range(N_CORES)), trace=trace
    )
    out_name = names["out"]
    parts = [
        res.results[c][out_name].reshape(B_LOC, J * C, S, S) for c in range(N_CORES)
    ]
    full = np.concatenate(parts, axis=0)
    return full, res


def kernel(**inputs):
    full, _ = kernel_with_results(trace=False, **inputs)
    return full


# revision 4
# speedup vs baseline: 1.1303x; 1.1303x over previous
"""Trainium2 Bass kernel for nn_Enhancement_11819749999257.

Computes: 3x (1x1-conv MLP w/ BN+relu) feature embeddings + soft scatter of
per-joint features onto a 7x7 grid ("bone projection"), concatenated.

Full output: (256, 4736, 7, 7) f32 = 237 MB  -> memory(write)-bound.

Strategy (pure data parallel over batch, 8 cores x 32 batch items):
  - n = b_local*74 + j  flattens (batch item, joint). The per-core output
    (32, 4736, 49) is contiguous as rows n: out[n, c*49+s]. Rows are
    processed in 19 chunks of 128 partitions; each chunk's store is a
    [128 part x 12544 B contiguous] DMA (~1.57 MB) -> near peak HBM BW.
  - MLP: w1/w2 are 64x64; BN (eval) folded into per-channel scale/bias on
    host. PE matmuls: y1 = relu(scale*(w1 @ x) + bias) in 5 column pieces;
    per chunk F = [y1_chunk; ones].T @ [w2.T; b2] (K=65 folds the b2 add)
    lands in PSUM in [n-partition, c-free] layout (no transpose needed).
  - Grid weights W[n, s] = relu(1 - sqrt((gy_s+eps-u_n)^2 + (gx_s+eps-v_n)^2))
    computed in 5 batched pieces: ACT Square (per-partition bias = -uv),
    one DVE add per piece, ACT Sqrt, ACT Relu.
  - Scatter: OUT[n, c*49+s] = F[n, c] * W[n, s] -- one DVE tensor_tensor
    mult per chunk with stride-0 broadcast APs ([128,64,1] x [128,1,49]).
  - Inputs stream on the SWDGE queue; output stores alternate between the
    two HWDGE rings (sync / scalar).
"""

import numpy as np

import concourse.bass as bass
import concourse.mybir as mybir
from concourse import bacc, bass_utils
from concourse.tile import TileContext

F32 = mybir.dt.float32
AF = mybir.ActivationFunctionType
ALU = mybir.AluOpType

N_CORES = 8
B = 256
B_LOC = B // N_CORES      # 32
J = 74                    # 21 + 21 + 32 joints, concat order r, l, o
C = 64
S = 7
S2 = S * S                # 49
NLOC = B_LOC * J          # 2368 rows per core
P = 128
NCHUNK = (NLOC + P - 1) // P   # 19 (last chunk has 64 valid rows)
NPAD = NCHUNK * P         # 2432
OUT_COLS = C * S2         # 3136
EPS = 1.0e-6
NA = 512                  # phase-A column piece
NPIECE = (NPAD + NA - 1) // NA  # 5


def _piece_chunks(a):
    return range(4 * a, min(4 * (a + 1), NCHUNK))


def _build_module():
    nc = bacc.Bacc(None)
    names = {}
    with TileContext(nc) as tc:
        with tc.tile_pool(name="dram", bufs=1, space="DRAM") as dram:
            xall = dram.tile((C, NPAD), F32, kind="ExternalInput", name="xall")
            ones = dram.tile((1, NPAD), F32, kind="ExternalInput", name="ones")
            nuv = dram.tile((P, 2 * NCHUNK), F32, kind="ExternalInput", name="nuv")
            gyc = dram.tile((P, S2), F32, kind="ExternalInput", name="gyc")
            gxc = dram.tile((P, S2), F32, kind="ExternalInput", name="gxc")
            w1t = dram.tile((C, C), F32, kind="ExternalInput", name="w1t")
            w2b = dram.tile((C + 1, C), F32, kind="ExternalInput", name="w2b")
            sc1 = dram.tile((C, 1), F32, kind="ExternalInput", name="sc1")
            bi1 = dram.tile((C, 1), F32, kind="ExternalInput", name="bi1")
            out = dram.tile((NLOC, OUT_COLS), F32, kind="ExternalOutput", name="out")
            for key, ap in (("xall", xall), ("ones", ones), ("nuv", nuv),
                            ("gyc", gyc), ("gxc", gxc), ("w1t", w1t),
                            ("w2b", w2b), ("sc1", sc1), ("bi1", bi1),
                            ("out", out)):
                names[key] = ap.tensor.name

            with (
                tc.tile_pool(name="consts", bufs=1) as cpool,
                tc.tile_pool(name="ps_a", bufs=2, space="PSUM") as ps_a,
                tc.tile_pool(name="ps_b", bufs=3, space="PSUM") as ps_b,
                tc.tile_pool(name="outs", bufs=6) as opool,
            ):
                pool_eng = mybir.EngineType.Pool
                nuv_sb = cpool.tile_from(nuv[:], forced_dma_engine=pool_eng)
                gyc_sb = cpool.tile_from(gyc[:], forced_dma_engine=pool_eng)
                gxc_sb = cpool.tile_from(gxc[:], forced_dma_engine=pool_eng)
                w1t_sb = cpool.tile_from(w1t[:], forced_dma_engine=pool_eng)
                w2b_sb = cpool.tile_from(w2b[:], forced_dma_engine=pool_eng)
                sc1_sb = cpool.tile_from(sc1[:], forced_dma_engine=pool_eng)
                bi1_sb = cpool.tile_from(bi1[:], forced_dma_engine=pool_eng)

                x_sb = cpool.tile((C, NPAD), F32)
                y1e = cpool.tile((C + 1, NPAD), F32)
                nc.gpsimd.dma_start(out=y1e[C : C + 1, :], in_=ones[:])

                # W pieces, batched: sq0/sq1/ss scratch, wv holds W[n, k*49+s]
                sq0 = cpool.tile((P, NCHUNK * S2), F32)
                sq1 = cpool.tile((P, NCHUNK * S2), F32)
                ss = cpool.tile((P, NCHUNK * S2), F32)
                wv = cpool.tile((P, NCHUNK * S2), F32)

                dma_out_engines = [nc.sync, nc.scalar]

                for a in range(NPIECE):
                    a0 = a * NA
                    aw = min(NA, NPAD - a0)
                    nc.gpsimd.dma_start(
                        out=x_sb[:, a0 : a0 + aw], in_=xall[:, a0 : a0 + aw]
                    )
                    ps1 = ps_a.tile((C, NA), F32, tag="ps1")
                    nc.tensor.matmul(
                        ps1[:, :aw], lhsT=w1t_sb, rhs=x_sb[:, a0 : a0 + aw]
                    )
                    nc.scalar.activation(
                        y1e[:C, a0 : a0 + aw], ps1[:, :aw], AF.Relu,
                        bias=bi1_sb, scale=sc1_sb,
                    )

                    # W for this piece's chunks
                    klo = 4 * a
                    kn = len(_piece_chunks(a))
                    for k in _piece_chunks(a):
                        nc.scalar.activation(
                            sq0[:, k * S2 : (k + 1) * S2], gyc_sb, AF.Square,
                            bias=nuv_sb[:, 2 * k : 2 * k + 1],
                        )
                        nc.scalar.activation(
                            sq1[:, k * S2 : (k + 1) * S2], gxc_sb, AF.Square,
                            bias=nuv_sb[:, 2 * k + 1 : 2 * k + 2],
                        )
                    psl = slice(klo * S2, (klo + kn) * S2)
                    nc.vector.tensor_tensor(ss[:, psl], sq0[:, psl], sq1[:, psl],
                                            ALU.add)
                    nc.scalar.activation(sq0[:, psl], ss[:, psl], AF.Sqrt)
                    nc.scalar.activation(wv[:, psl], sq0[:, psl], AF.Relu,
                                         bias=1.0, scale=-1.0)

                    for k in _piece_chunks(a):
                        rows = min(P, NLOC - k * P)
                        # F = [y1;1].T @ [w2t;b2] -> PSUM [128 rows(n), 64 (c)]
                        psf = ps_b.tile((P, C), F32, tag="psf")
                        nc.tensor.matmul(
                            psf[:], lhsT=y1e[:, k * P : (k + 1) * P], rhs=w2b_sb
                        )
                        # OUT[n, c*49+s] = F[n, c] * W[n, s]
                        o_sb = opool.tile((P, OUT_COLS), F32, tag="o")
                        f_bc, w_bc = bass.broadcast_tensor_aps(
                            psf[:, :, None], wv[:, k * S2 : (k + 1) * S2][:, None, :]
                        )
                        o_3d = o_sb.rearrange("p (c s) -> p c s", s=S2)
                        nc.vector.tensor_tensor(o_3d, f_bc, w_bc, ALU.mult)
                        dma_out_engines[k % 2].dma_start(
                            out=out[k * P : k * P + rows, :], in_=o_sb[:rows, :]
                        )
    nc.compile()
    return nc, names


_CACHE = {}


def _get_module():
    if "nc" not in _CACHE:
        _CACHE["nc"], _CACHE["names"] = _build_module()
    return _CACHE["nc"], _CACHE["names"]


def _prep_inputs(j2d_r, j2d_l, kp2d_o, feat_r, feat_l, feat_o,
                 w1, b1, bn_gamma, bn_beta, bn_mean, bn_var, w2, b2):
    """Host-side marshaling: shard batch, pack layouts. Returns in_maps."""
    f32 = np.float32
    # grid: grid[s] = (x[s%7], x[s//7]) with x = arange(7)+0.5
    x = (np.arange(S, dtype=f32) + 0.5)
    gy = np.tile(x, S) + EPS            # gy[s] = x[s%7] + eps
    gx = np.repeat(x, S) + EPS          # gx[s] = x[s//7] + eps
    gyc = np.broadcast_to(gy, (P, S2)).copy()
    gxc = np.broadcast_to(gx, (P, S2)).copy()

    scale = (bn_gamma.astype(f32) / np.sqrt(bn_var.astype(f32) + np.float32(1e-5)))
    bias1 = (b1.astype(f32) - bn_mean.astype(f32)) * scale + bn_beta.astype(f32)
    w1t = np.ascontiguousarray(w1.astype(f32).T)
    w2b = np.concatenate(
        [w2.astype(f32).T, b2.astype(f32).reshape(1, C)], axis=0
    )  # (65, 64)
    ones = np.ones((1, NPAD), f32)

    xcat = np.concatenate([feat_r, feat_l, feat_o], axis=2).astype(f32)  # (B,64,74)
    jcat = np.concatenate([j2d_r, j2d_l, kp2d_o], axis=1).astype(f32)   # (B,74,2)

    shared = dict(
        ones=ones, gyc=gyc, gxc=gxc, w1t=w1t, w2b=w2b,
        sc1=scale.reshape(C, 1).copy(), bi1=bias1.reshape(C, 1).copy(),
    )
    in_maps = []
    for c in range(N_CORES):
        sl = slice(c * B_LOC, (c + 1) * B_LOC)
        # xall[c_ch, n] = xcat[b', c_ch, j], n = b'*74+j ; pad n to 2432
        xc = np.transpose(xcat[sl], (1, 0, 2)).reshape(C, NLOC)
        xall = np.zeros((C, NPAD), f32)
        xall[:, :NLOC] = xc
        # nuv[p, 2k+i] = -(jcat[n=128k+p, i] + 1) * 3.5
        jc = np.zeros((NPAD, 2), f32)
        jc[:NLOC] = jcat[sl].reshape(NLOC, 2)
        nuv_flat = -(jc + np.float32(1.0)) * np.float32(3.5)
        nuv = np.ascontiguousarray(
            nuv_flat.reshape(NCHUNK, P, 2).transpose(1, 0, 2).reshape(P, 2 * NCHUNK)
        )
        in_maps.append(dict(shared, xall=xall, nuv=nuv))
    return in_maps


def kernel_with_results(trace=False, **inputs):
    nc, names = _get_module()
    in_maps_l = _prep_inputs(**inputs)
    in_maps = [{names[k]: v for k, v in m.items()} for m in in_maps_l]
    res = bass_utils.run_bass_kernel_spmd(
        nc, in_maps, core_ids=list(range(N_CORES)), trace=trace
    )
    out_name = names["out"]
    parts = [
        res.results[c][out_name].reshape(B_LOC, J * C, S, S) for c in range(N_CORES)
    ]
    full = np.concatenate(parts, axis=0)
    return full, res


def kernel(**inputs):
    full, _ = kernel_with_results(trace=False, **inputs)
    return full


# revision 8
# speedup vs baseline: 1.1987x; 1.0605x over previous
"""Trainium2 Bass kernel for nn_Enhancement_11819749999257.

Computes: 3x (1x1-conv MLP w/ BN+relu) feature embeddings + soft scatter of
per-joint features onto a 7x7 grid ("bone projection"), concatenated.

Full output: (256, 4736, 7, 7) f32 = 237 MB  -> memory(write)-bound.

Strategy (pure data parallel over batch, 8 cores x 32 batch items):
  - n = b_local*74 + j  flattens (batch item, joint). The per-core output
    (32, 4736, 49) is contiguous as rows n: out[n, c*49+s]. Rows are
    processed in 19 chunks of 128 partitions; each chunk's store is a
    [128 part x 12544 B contiguous] DMA (~1.57 MB) -> near peak HBM BW.
  - MLP: w1/w2 are 64x64; BN (eval) folded into per-channel scale/bias on
    host. PE matmuls: y1 = relu(scale*(w1 @ x) + bias) in 5 column pieces;
    per chunk F = [y1_chunk; ones].T @ [w2.T; b2] (K=65 folds the b2 add)
    lands in PSUM in [n-partition, c-free] layout (no transpose needed).
  - Grid weights W[n, s] = relu(1 - sqrt((gy_s+eps-u_n)^2 + (gx_s+eps-v_n)^2))
    computed in 5 batched pieces: ACT Square (per-partition bias = -uv),
    one DVE add per piece, ACT Sqrt, ACT Relu.
  - Scatter: OUT[n, c*49+s] = F[n, c] * W[n, s] -- one DVE tensor_tensor
    mult per chunk with stride-0 broadcast APs ([128,64,1] x [128,1,49]).
  - Inputs stream on the SWDGE queue; output stores alternate between the
    two HWDGE rings (sync / scalar).
"""

import numpy as np

import concourse.bass as bass
import concourse.mybir as mybir
from concourse import bacc, bass_utils
from concourse.tile import TileContext

F32 = mybir.dt.float32
AF = mybir.ActivationFunctionType
ALU = mybir.AluOpType

N_CORES = 8
B = 256
B_LOC = B // N_CORES      # 32
J = 74                    # 21 + 21 + 32 joints, concat order r, l, o
C = 64
S = 7
S2 = S * S                # 49
NLOC = B_LOC * J          # 2368 rows per core
P = 128
NCHUNK = (NLOC + P - 1) // P   # 19 (last chunk has 64 valid rows)
NPAD = NCHUNK * P         # 2432
OUT_COLS = C * S2         # 3136
EPS = 1.0e-6
NA = 512                  # phase-A column piece
NPIECE = (NPAD + NA - 1) // NA  # 5

# packed-constants column layout: [gyc|gxc|nuv|w1t|w2b|sc1|bi1]
OFF_GY = 0
OFF_GX = OFF_GY + S2            # 49
OFF_NUV = OFF_GX + S2           # 98
OFF_W1 = OFF_NUV + 2 * NCHUNK   # 136
OFF_W2B = OFF_W1 + C            # 200
OFF_SC = OFF_W2B + C            # 264
OFF_BI = OFF_SC + 1             # 265
NCONST = OFF_BI + 1             # 266


def _piece_chunks(a):
    return range(4 * a, min(4 * (a + 1), NCHUNK))


def _build_module():
    nc = bacc.Bacc(None)
    names = {}
    with TileContext(nc) as tc:
        with tc.tile_pool(name="dram", bufs=1, space="DRAM") as dram:
            xall = dram.tile((C, NPAD), F32, kind="ExternalInput", name="xall")
            cpk = dram.tile((P, NCONST), F32, kind="ExternalInput", name="cpk")
            out = dram.tile((NLOC, OUT_COLS), F32, kind="ExternalOutput", name="out")
            for key, ap in (("xall", xall), ("cpk", cpk), ("out", out)):
                names[key] = ap.tensor.name

            with (
                tc.tile_pool(name="consts", bufs=1) as cpool,
                tc.tile_pool(name="ps_a", bufs=2, space="PSUM") as ps_a,
                tc.tile_pool(name="ps_b", bufs=3, space="PSUM") as ps_b,
                tc.tile_pool(name="outs", bufs=8) as opool,
            ):
                cpk_sb = cpool.tile((P, NCONST), F32)
                nc.sync.dma_start(out=cpk_sb[:], in_=cpk[:])
                gyc_sb = cpk_sb[:, OFF_GY : OFF_GY + S2]
                gxc_sb = cpk_sb[:, OFF_GX : OFF_GX + S2]
                nuv_sb = cpk_sb[:, OFF_NUV : OFF_NUV + 2 * NCHUNK]
                w1t_sb = cpk_sb[:C, OFF_W1 : OFF_W1 + C]
                w2b_sb = cpk_sb[: C + 1, OFF_W2B : OFF_W2B + C]
                sc1_sb = cpk_sb[:C, OFF_SC : OFF_SC + 1]
                bi1_sb = cpk_sb[:C, OFF_BI : OFF_BI + 1]

                x_sb = cpool.tile((C, NPAD), F32)
                y1e = cpool.tile((C + 1, NPAD), F32)
                nc.gpsimd.memset(y1e[C : C + 1, :], 1.0)

                # W pieces, batched: sq0/sq1/ss scratch, wv holds W[n, k*49+s]
                sq0 = cpool.tile((P, NCHUNK * S2), F32)
                sq1 = cpool.tile((P, NCHUNK * S2), F32)
                ss = cpool.tile((P, NCHUNK * S2), F32)
                wv = cpool.tile((P, NCHUNK * S2), F32)

                dma_out_engines = [nc.sync, nc.scalar]

                for a in range(NPIECE):
                    a0 = a * NA
                    aw = min(NA, NPAD - a0)
                    nc.scalar.dma_start(
                        out=x_sb[:, a0 : a0 + aw], in_=xall[:, a0 : a0 + aw]
                    )
                    ps1 = ps_a.tile((C, NA), F32, tag="ps1")
                    nc.tensor.matmul(
                        ps1[:, :aw], lhsT=w1t_sb, rhs=x_sb[:, a0 : a0 + aw]
                    )
                    nc.scalar.activation(
                        y1e[:C, a0 : a0 + aw], ps1[:, :aw], AF.Relu,
                        bias=bi1_sb, scale=sc1_sb,
                    )

                    # W for this piece's chunks
                    klo = 4 * a
                    kn = len(_piece_chunks(a))
                    for k in _piece_chunks(a):
                        nc.scalar.activation(
                            sq0[:, k * S2 : (k + 1) * S2], gyc_sb, AF.Square,
                            bias=nuv_sb[:, 2 * k : 2 * k + 1],
                        )
                        nc.scalar.activation(
                            sq1[:, k * S2 : (k + 1) * S2], gxc_sb, AF.Square,
                            bias=nuv_sb[:, 2 * k + 1 : 2 * k + 2],
                        )
                    psl = slice(klo * S2, (klo + kn) * S2)
                    nc.vector.tensor_tensor(ss[:, psl], sq0[:, psl], sq1[:, psl],
                                            ALU.add)
                    nc.scalar.activation(sq0[:, psl], ss[:, psl], AF.Sqrt)
                    nc.scalar.activation(wv[:, psl], sq0[:, psl], AF.Relu,
                                         bias=1.0, scale=-1.0)

                    for k in _piece_chunks(a):
                        rows = min(P, NLOC - k * P)
                        # F = [y1;1].T @ [w2t;b2] -> PSUM [128 rows(n), 64 (c)]
                        psf = ps_b.tile((P, C), F32, tag="psf")
                        nc.tensor.matmul(
                            psf[:], lhsT=y1e[:, k * P : (k + 1) * P], rhs=w2b_sb
                        )
                        # OUT[n, c*49+s] = F[n, c] * W[n, s]
                        o_sb = opool.tile((P, OUT_COLS), F32, tag="o")
                        f_bc, w_bc = bass.broadcast_tensor_aps(
                            psf[:, :, None], wv[:, k * S2 : (k + 1) * S2][:, None, :]
                        )
                        o_3d = o_sb.rearrange("p (c s) -> p c s", s=S2)
                        nc.vector.tensor_tensor(o_3d, f_bc, w_bc, ALU.mult)
                        dma_out_engines[k % 2].dma_start(
                            out=out[k * P : k * P + rows, :], in_=o_sb[:rows, :]
                        )
    nc.compile()
    return nc, names


_CACHE = {}


def _get_module():
    if "nc" not in _CACHE:
        _CACHE["nc"], _CACHE["names"] = _build_module()
    return _CACHE["nc"], _CACHE["names"]


def _prep_inputs(j2d_r, j2d_l, kp2d_o, feat_r, feat_l, feat_o,
                 w1, b1, bn_gamma, bn_beta, bn_mean, bn_var, w2, b2):
    """Host-side marshaling: shard batch, pack layouts. Returns in_maps."""
    f32 = np.float32
    # grid: grid[s] = (x[s%7], x[s//7]) with x = arange(7)+0.5
    x = (np.arange(S, dtype=f32) + 0.5)
    gy = np.tile(x, S) + EPS            # gy[s] = x[s%7] + eps
    gx = np.repeat(x, S) + EPS          # gx[s] = x[s//7] + eps
    gyc = np.broadcast_to(gy, (P, S2)).copy()
    gxc = np.broadcast_to(gx, (P, S2)).copy()

    scale = (bn_gamma.astype(f32) / np.sqrt(bn_var.astype(f32) + np.float32(1e-5)))
    bias1 = (b1.astype(f32) - bn_mean.astype(f32)) * scale + bn_beta.astype(f32)

    cpk0 = np.zeros((P, NCONST), f32)
    cpk0[:, OFF_GY : OFF_GY + S2] = gyc
    cpk0[:, OFF_GX : OFF_GX + S2] = gxc
    cpk0[:C, OFF_W1 : OFF_W1 + C] = w1.astype(f32).T
    cpk0[:C, OFF_W2B : OFF_W2B + C] = w2.astype(f32).T
    cpk0[C, OFF_W2B : OFF_W2B + C] = b2.astype(f32)
    cpk0[:C, OFF_SC] = scale
    cpk0[:C, OFF_BI] = bias1

    xcat = np.concatenate([feat_r, feat_l, feat_o], axis=2).astype(f32)  # (B,64,74)
    jcat = np.concatenate([j2d_r, j2d_l, kp2d_o], axis=1).astype(f32)   # (B,74,2)

    in_maps = []
    for c in range(N_CORES):
        sl = slice(c * B_LOC, (c + 1) * B_LOC)
        # xall[c_ch, n] = xcat[b', c_ch, j], n = b'*74+j ; pad n to 2432
        xc = np.transpose(xcat[sl], (1, 0, 2)).reshape(C, NLOC)
        xall = np.zeros((C, NPAD), f32)
        xall[:, :NLOC] = xc
        # nuv[p, 2k+i] = -(jcat[n=128k+p, i] + 1) * 3.5
        jc = np.zeros((NPAD, 2), f32)
        jc[:NLOC] = jcat[sl].reshape(NLOC, 2)
        nuv_flat = -(jc + np.float32(1.0)) * np.float32(3.5)
        cpk = cpk0.copy()
        cpk[:, OFF_NUV : OFF_NUV + 2 * NCHUNK] = (
            nuv_flat.reshape(NCHUNK, P, 2).transpose(1, 0, 2).reshape(P, 2 * NCHUNK)
        )
        in_maps.append(dict(xall=xall, cpk=cpk))
    return in_maps


def kernel_with_results(trace=False, **inputs):
    nc, names = _get_module()
    in_maps_l = _prep_inputs(**inputs)
    in_maps = [{names[k]: v for k, v in m.items()} for m in in_maps_l]
    res = bass_utils.run_bass_kernel_spmd(
        nc, in_maps, core_ids=list(range(N_CORES)), trace=trace
    )
    out_name = names["out"]
    parts = [
        res.results[c][out_name].reshape(B_LOC, J * C, S, S) for c in range(N_CORES)
    ]
    full = np.concatenate(parts, axis=0)
    return full, res


def kernel(**inputs):
    full, _ = kernel_with_results(trace=False, **inputs)
    return full
